# revision 32
# baseline (speedup 1.0000x reference)
"""Distributed k-NN retrieval (MemoryBank) on 8 Trainium2 NeuronCores.

Strategy (memory rows sharded 8 ways, queries replicated):
  Device (per core):
    - normalize its memory shard rows (1/max(|m|,eps)), cast bf16,
      DMA-transpose to [D, M] layout; cast+transpose queries (not normalized:
      a per-query positive scale never changes that query's ranking).
    - 32 query tiles x 26 matmul chunks (N=512) -> PSUM f32 sims.
    - max-accumulate drains per DRAIN_PLAN, split between DVE (reads PSUM
      directly) and ScalarE (cast-copies PSUM->SBUF bf16, DVE folds at 2x) to
      balance the two engines; each route keeps its own [128, 2048] bf16
      accumulator. One engine-read per PSUM element is the hard floor.
    - ship [4096, 2*2048] bf16 group-max matrix per core to host.
  Host:
    - top-6 groups per query across all cores (top-3 groups provably contain
      the true top-3 values), rescore <=42 candidate rows exactly in fp32,
      emit top-k (distances = 1-sims, indices), ties -> lowest index.
"""

import functools

import numpy as np

# ---- hardcoded problem geometry (self-contained; do not read spec files) ----
NQ = 4096          # queries
D = 128            # feature dim
M_TOTAL = 100000   # memory rows
N_CORES = 8
M_SHARD = 13312    # padded per-core rows = 104*128 = 13*1024
M_PAD_TOTAL = M_SHARD * N_CORES
NQT = NQ // 128    # 32 query tiles
N_MTILE = M_SHARD // 128  # 104
EPS = 1e-12

# number of top groups rescored on host (3 suffices in exact arithmetic;
# extra groups absorb bf16 rounding ties)
T_GROUPS = 6

# Drain plan: PSUM tiles of width w starting at memory-column base, each
# drained by route 'D' (DVE reads PSUM directly, 1x) or 'A' (ScalarE
# cast-copies PSUM->SBUF bf16; DVE folds in bf16 at 2x). Each route owns a
# [128, ACC_W] bf16 accumulator; entry (w, r, base) merges psum[:, :w] into
# acc_r[:, :w]. Host resolves group (r, u) -> candidate rows {base + u}.
# GpSimd has no TENSOR_TENSOR opcode on TRN2; ScalarE cannot max.
ACC_W = 2048
DRAIN_PLAN = [
    (2048, "A", 0),
    (2048, "A", 2048),
    (2048, "D", 4096),
    (2048, "A", 6144),
    (2048, "A", 8192),
    (2048, "A", 10240),
    (1024, "D", 12288),
]
assert sum(w for w, _, _ in DRAIN_PLAN) == M_SHARD
ROUTE_NAMES = "DA"
N_ROUTES = len(ROUTE_NAMES)


def _group_members():
    """[N_ROUTES, ACC_W, max_members] candidate local-row table, -1 padded."""
    lists = [[[] for _ in range(ACC_W)] for _ in ROUTE_NAMES]
    for w, r, base in DRAIN_PLAN:
        ri = ROUTE_NAMES.index(r)
        for u in range(w):
            lists[ri][u].append(base + u)
    mm = max(len(x) for l in lists for x in l)
    arr = np.full((N_ROUTES, ACC_W, mm), -1, dtype=np.int64)
    for ri in range(N_ROUTES):
        for u in range(ACC_W):
            arr[ri, u, :len(lists[ri][u])] = lists[ri][u]
    return arr


@functools.lru_cache(maxsize=1)
def _build_nc():
    import concourse.mybir as mybir
    from concourse import bacc, tile

    f32 = mybir.dt.float32
    bf16 = mybir.dt.bfloat16
    AF = mybir.ActivationFunctionType
    MAX = mybir.AluOpType.max
    AX = mybir.AxisListType.X

    nc = bacc.Bacc("TRN2", target_bir_lowering=False, debug=False)

    mem_in = nc.dram_tensor("mem", [M_SHARD, D], f32, kind="ExternalInput")
    q_in = nc.dram_tensor("queries", [NQ, D], f32, kind="ExternalInput")
    id_in = nc.dram_tensor("ident", [128, 128], bf16, kind="ExternalInput")
    cm_out = nc.dram_tensor(
        "cm", [NQ, N_ROUTES * ACC_W], bf16, kind="ExternalOutput")

    with tile.TileContext(nc) as tc:
        with (
            tc.tile_pool(name="const", bufs=1) as const_pool,
            tc.tile_pool(name="stage", bufs=1) as stage_pool,
            tc.tile_pool(name="prep", bufs=2) as prep_pool,
            tc.tile_pool(name="work", bufs=2) as work_pool,
        ):
            mT = const_pool.tile([128, M_SHARD], bf16, tag="mT")
            qT = const_pool.tile([128, NQ], bf16, tag="qT")
            ident = const_pool.tile([128, 128], bf16, tag="ident")
            nc.sync.dma_start(ident[:], id_in.ap())

            # prep uses its own PSUM scope (closed before the main loop so the
            # main PSUM pool can use all 8 banks)
            with tc.tile_pool(name="tpsum", bufs=2, space="PSUM") as tpsum_pool:
                # ---------------- prep: queries -> qT (bf16, transposed) ----
                qstage = stage_pool.tile([128, NQT * D], f32, tag="qstage")
                nc.sync.dma_start(
                    qstage[:].rearrange("p (t d) -> p t d", d=D),
                    q_in.ap().rearrange("(t p) d -> p t d", p=128),
                )
                identf = const_pool.tile([128, 128], f32, tag="identf")
                nc.scalar.copy(identf[:], ident[:])
                for t in range(NQT):
                    tp = tpsum_pool.tile([128, 128], f32, tag="tp")
                    nc.tensor.transpose(
                        tp[:], qstage[:, t * D:(t + 1) * D], identf[:])
                    nc.vector.tensor_copy(qT[:, t * 128:(t + 1) * 128], tp[:])

                # ------------- prep: memory -> mT (normalized bf16, T) ------
                ss = const_pool.tile([128, N_MTILE], f32, tag="ss")
                N_PIECE = 4
                TPP = N_MTILE // N_PIECE  # 26 tiles per piece
                for piece in range(N_PIECE):
                    mstage = stage_pool.tile(
                        [128, TPP * D], f32, tag=f"mstage{piece}",
                        name=f"mstage{piece}")
                    r0 = piece * TPP * 128
                    nc.sync.dma_start(
                        mstage[:].rearrange("p (t d) -> p t d", d=D),
                        mem_in.ap()[r0:r0 + TPP * 128, :].rearrange(
                            "(t p) d -> p t d", p=128),
                    )
                    sq = prep_pool.tile([128, TPP * D], f32, tag="sq")
                    nc.scalar.activation(sq[:], mstage[:], AF.Square)
                    nc.vector.reduce_sum(
                        ss[:, piece * TPP:(piece + 1) * TPP],
                        sq[:].rearrange("p (t d) -> p t d", d=D),
                        axis=AX,
                    )
                    # normalize (DVE 2x f32->bf16) + PE transpose into mT
                    m_bf = prep_pool.tile([128, TPP * D], bf16, tag="m_bf")
                    norm = prep_pool.tile([128, TPP], f32, tag="norm")
                    scale = prep_pool.tile([128, TPP], f32, tag="scale")
                    nc.scalar.activation(
                        norm[:], ss[:, piece * TPP:(piece + 1) * TPP], AF.Sqrt)
                    nc.vector.tensor_scalar_max(norm[:], norm[:], EPS)
                    nc.vector.reciprocal(scale[:], norm[:])
                    for t in range(TPP):
                        nc.vector.tensor_scalar_mul(
                            m_bf[:, t * D:(t + 1) * D],
                            mstage[:, t * D:(t + 1) * D],
                            scale[:, t:t + 1],
                        )
                    for t in range(TPP):
                        tg = piece * TPP + t
                        tp = tpsum_pool.tile([128, 128], bf16, tag="tp")
                        nc.tensor.transpose(
                            tp[:], m_bf[:, t * D:(t + 1) * D], ident[:])
                        nc.scalar.copy(mT[:, tg * 128:(tg + 1) * 128], tp[:])

            # ---------------- main: sims + routed max-accumulate drains -----
            with tc.tile_pool(name="psum", bufs=2, space="PSUM") as psum_pool:
                for qt in range(NQT):
                    accs = {r: work_pool.tile([128, ACC_W], bf16,
                                              tag=f"acc{r}", name=f"acc{r}")
                            for r in ROUTE_NAMES}
                    seen = {r: False for r in ROUTE_NAMES}
                    lhsT = qT[:, qt * 128:(qt + 1) * 128]
                    for w, r, base in DRAIN_PLAN:
                        ps = psum_pool.tile([128, ACC_W], f32, tag="ps")
                        for j in range(w // 512):
                            nc.tensor.matmul(
                                ps[:, j * 512:(j + 1) * 512], lhsT,
                                mT[:, base + j * 512:base + (j + 1) * 512],
                                start=True, stop=True,
                            )
                        acc = accs[r]
                        if r == "D":
                            if not seen[r]:
                                nc.vector.tensor_copy(acc[:, :w], ps[:, :w])
                            else:
                                nc.vector.tensor_tensor(
                                    acc[:, :w], ps[:, :w], acc[:, :w], op=MAX)
                        elif not seen[r]:
                            nc.scalar.copy(acc[:, :w], ps[:, :w])
                        else:
                            tmp = work_pool.tile([128, ACC_W], bf16,
                                                 tag="tmpA", name="tmpA")
                            nc.scalar.copy(tmp[:, :w], ps[:, :w])
                            nc.vector.tensor_tensor(
                                acc[:, :w], tmp[:, :w], acc[:, :w], op=MAX)
                        seen[r] = True
                    for ri, r in enumerate(ROUTE_NAMES):
                        nc.sync.dma_start(
                            cm_out.ap()[qt * 128:(qt + 1) * 128,
                                        ri * ACC_W:(ri + 1) * ACC_W],
                            accs[r][:],
                        )

    nc.compile()
    return nc


def _identity_bf16():
    import ml_dtypes

    return np.eye(128, dtype=ml_dtypes.bfloat16)


def _in_maps(queries_np, mem_padded):
    shards = mem_padded.reshape(N_CORES, M_SHARD, D)
    ident = _identity_bf16()
    return [
        {"mem": np.ascontiguousarray(shards[c]), "queries": queries_np,
         "ident": ident}
        for c in range(N_CORES)
    ]


def _run_device(queries_np, mem_padded, trace=False):
    from concourse import bass_utils

    nc = _build_nc()
    res = bass_utils.run_bass_kernel_spmd(
        nc, _in_maps(queries_np, mem_padded),
        core_ids=list(range(N_CORES)), trace=trace,
    )
    return res


def _host_topk(queries_np, memory_np, cm_all, k):
    import ml_dtypes  # noqa: F401  (cm arrives as bfloat16)

    nq = queries_np.shape[0]
    # [NQ, N_CORES * N_ROUTES * ACC_W] routed group-max matrix
    cm = np.concatenate(
        [np.asarray(cm_all[c], dtype=np.float32) for c in range(N_CORES)], axis=1
    )
    t = min(T_GROUPS, cm.shape[1])
    top_groups = np.argpartition(-cm, t - 1, axis=1)[:, :t]  # [NQ, t]

    per_core = N_ROUTES * ACC_W
    core = top_groups // per_core
    rem = top_groups % per_core
    ri = rem // ACC_W
    u = rem % ACC_W
    members = _group_members()                       # [N_ROUTES, ACC_W, mm]
    loc = members[ri, u]                             # [NQ, t, mm]
    cand = (core[:, :, None] * M_SHARD + loc).reshape(nq, -1)
    cand = np.where(loc.reshape(nq, -1) < 0, M_PAD_TOTAL, cand)  # pad slots

    valid = cand < M_TOTAL
    cand_safe = np.where(valid, cand, 0)

    qn = queries_np / np.maximum(
        np.linalg.norm(queries_np, axis=1, keepdims=True), EPS)
    mc = memory_np[cand_safe]                             # [NQ, t*16, D]
    mc_n = np.linalg.norm(mc, axis=2, keepdims=True)
    mc = mc / np.maximum(mc_n, EPS)
    vals = np.einsum("qd,qcd->qc", qn.astype(np.float32), mc.astype(np.float32))
    vals = np.where(valid, vals, np.float32(-2.0))

    # sort candidates by index so a stable sort on -vals breaks ties by index
    ordc = np.argsort(cand_safe, axis=1)
    cand_sorted = np.take_along_axis(cand_safe, ordc, axis=1)
    vals_sorted = np.take_along_axis(vals, ordc, axis=1)
    sel = np.argsort(-vals_sorted, axis=1, kind="stable")[:, :k]

    top_vals = np.take_along_axis(vals_sorted, sel, axis=1)
    top_idx = np.take_along_axis(cand_sorted, sel, axis=1)
    distances = (np.float32(1.0) - top_vals).astype(np.float32)
    indices = top_idx.astype(np.int32)
    return distances, indices


def kernel(queries, memory, k):
    queries_np = np.ascontiguousarray(np.asarray(queries, dtype=np.float32))
    memory_np = np.ascontiguousarray(np.asarray(memory, dtype=np.float32))
    k = int(np.asarray(k))

    mem_padded = np.zeros((M_PAD_TOTAL, D), dtype=np.float32)
    mem_padded[:M_TOTAL] = memory_np

    res = _run_device(queries_np, mem_padded)
    cm_all = [res.results[c]["cm"] for c in range(N_CORES)]
    return _host_topk(queries_np, memory_np, cm_all, k)


# revision 36
# speedup vs baseline: 1.0749x; 1.0749x over previous
"""Distributed k-NN retrieval (MemoryBank) on 8 Trainium2 NeuronCores.

Strategy (memory rows sharded 8 ways, queries replicated):
  Device (per core):
    - normalize its memory shard rows (1/max(|m|,eps)), cast bf16,
      DMA-transpose to [D, M] layout; cast+transpose queries (not normalized:
      a per-query positive scale never changes that query's ranking).
    - 32 query tiles x 26 matmul chunks (N=512) -> PSUM f32 sims.
    - max-accumulate drains per DRAIN_PLAN, split between DVE (reads PSUM
      directly) and ScalarE (cast-copies PSUM->SBUF bf16, DVE folds at 2x) to
      balance the two engines; each route keeps its own [128, 2048] bf16
      accumulator. One engine-read per PSUM element is the hard floor.
    - ship [4096, 2*2048] bf16 group-max matrix per core to host.
  Host:
    - top-6 groups per query across all cores (top-3 groups provably contain
      the true top-3 values), rescore <=42 candidate rows exactly in fp32,
      emit top-k (distances = 1-sims, indices), ties -> lowest index.
"""

import functools

import numpy as np

# ---- hardcoded problem geometry (self-contained; do not read spec files) ----
NQ = 4096          # queries
D = 128            # feature dim
M_TOTAL = 100000   # memory rows
N_CORES = 8
M_SHARD = 13312    # padded per-core rows = 104*128 = 13*1024
M_PAD_TOTAL = M_SHARD * N_CORES
NQT = NQ // 128    # 32 query tiles
N_MTILE = M_SHARD // 128  # 104
EPS = 1e-12

# number of top groups rescored on host (3 suffices in exact arithmetic;
# extra groups absorb bf16 rounding ties)
T_GROUPS = 6

# Drain plan: PSUM tiles of width w starting at memory-column base, each
# drained by route 'D' (DVE reads PSUM directly, 1x) or 'A' (ScalarE
# cast-copies PSUM->SBUF bf16; DVE folds in bf16 at 2x). Each route owns a
# [128, ACC_W] bf16 accumulator; entry (w, r, base) merges psum[:, :w] into
# acc_r[:, :w]. Host resolves group (r, u) -> candidate rows {base + u}.
# GpSimd has no TENSOR_TENSOR opcode on TRN2; ScalarE cannot max.
ACC_W = 1024
_ROUTE_PATTERN = "DAADAADAADAAA"
DRAIN_PLAN = [
    (1024, _ROUTE_PATTERN[i], 1024 * i) for i in range(M_SHARD // 1024)
]
assert sum(w for w, _, _ in DRAIN_PLAN) == M_SHARD
ROUTE_NAMES = "DA"
N_ROUTES = len(ROUTE_NAMES)


def _group_members():
    """[N_ROUTES, ACC_W, max_members] candidate local-row table, -1 padded."""
    lists = [[[] for _ in range(ACC_W)] for _ in ROUTE_NAMES]
    for w, r, base in DRAIN_PLAN:
        ri = ROUTE_NAMES.index(r)
        for u in range(w):
            lists[ri][u].append(base + u)
    mm = max(len(x) for l in lists for x in l)
    arr = np.full((N_ROUTES, ACC_W, mm), -1, dtype=np.int64)
    for ri in range(N_ROUTES):
        for u in range(ACC_W):
            arr[ri, u, :len(lists[ri][u])] = lists[ri][u]
    return arr


@functools.lru_cache(maxsize=1)
def _build_nc():
    import concourse.mybir as mybir
    from concourse import bacc, tile

    f32 = mybir.dt.float32
    bf16 = mybir.dt.bfloat16
    AF = mybir.ActivationFunctionType
    MAX = mybir.AluOpType.max
    AX = mybir.AxisListType.X

    nc = bacc.Bacc("TRN2", target_bir_lowering=False, debug=False)

    mem_in = nc.dram_tensor("mem", [M_SHARD, D], f32, kind="ExternalInput")
    q_in = nc.dram_tensor("queries", [NQ, D], f32, kind="ExternalInput")
    id_in = nc.dram_tensor("ident", [128, 128], bf16, kind="ExternalInput")
    cm_out = nc.dram_tensor(
        "cm", [NQ, N_ROUTES * ACC_W], bf16, kind="ExternalOutput")

    with tile.TileContext(nc) as tc:
        with (
            tc.tile_pool(name="const", bufs=1) as const_pool,
            tc.tile_pool(name="stage", bufs=1) as stage_pool,
            tc.tile_pool(name="prep", bufs=2) as prep_pool,
            tc.tile_pool(name="work", bufs=2) as work_pool,
        ):
            mT = const_pool.tile([128, M_SHARD], bf16, tag="mT")
            qT = const_pool.tile([128, NQ], bf16, tag="qT")
            ident = const_pool.tile([128, 128], bf16, tag="ident")
            nc.sync.dma_start(ident[:], id_in.ap())

            # prep uses its own PSUM scope (closed before the main loop so the
            # main PSUM pool can use all 8 banks)
            with tc.tile_pool(name="tpsum", bufs=2, space="PSUM") as tpsum_pool:
                # ---------------- prep: queries -> qT (bf16, transposed) ----
                qstage = stage_pool.tile([128, NQT * D], f32, tag="qstage")
                nc.sync.dma_start(
                    qstage[:].rearrange("p (t d) -> p t d", d=D),
                    q_in.ap().rearrange("(t p) d -> p t d", p=128),
                )
                identf = const_pool.tile([128, 128], f32, tag="identf")
                nc.scalar.copy(identf[:], ident[:])
                for t in range(NQT):
                    tp = tpsum_pool.tile([128, 128], f32, tag="tp")
                    nc.tensor.transpose(
                        tp[:], qstage[:, t * D:(t + 1) * D], identf[:])
                    nc.vector.tensor_copy(qT[:, t * 128:(t + 1) * 128], tp[:])

                # ------------- prep: memory -> mT (normalized bf16, T) ------
                ss = const_pool.tile([128, N_MTILE], f32, tag="ss")
                N_PIECE = 4
                TPP = N_MTILE // N_PIECE  # 26 tiles per piece
                for piece in range(N_PIECE):
                    mstage = stage_pool.tile(
                        [128, TPP * D], f32, tag=f"mstage{piece}",
                        name=f"mstage{piece}")
                    r0 = piece * TPP * 128
                    nc.sync.dma_start(
                        mstage[:].rearrange("p (t d) -> p t d", d=D),
                        mem_in.ap()[r0:r0 + TPP * 128, :].rearrange(
                            "(t p) d -> p t d", p=128),
                    )
                    sq = prep_pool.tile([128, TPP * D], f32, tag="sq")
                    nc.scalar.activation(sq[:], mstage[:], AF.Square)
                    nc.vector.reduce_sum(
                        ss[:, piece * TPP:(piece + 1) * TPP],
                        sq[:].rearrange("p (t d) -> p t d", d=D),
                        axis=AX,
                    )
                    # normalize (DVE 2x f32->bf16) + PE transpose into mT
                    m_bf = prep_pool.tile([128, TPP * D], bf16, tag="m_bf")
                    norm = prep_pool.tile([128, TPP], f32, tag="norm")
                    scale = prep_pool.tile([128, TPP], f32, tag="scale")
                    nc.scalar.activation(
                        norm[:], ss[:, piece * TPP:(piece + 1) * TPP], AF.Sqrt)
                    nc.vector.tensor_scalar_max(norm[:], norm[:], EPS)
                    nc.vector.reciprocal(scale[:], norm[:])
                    for t in range(TPP):
                        nc.vector.tensor_scalar_mul(
                            m_bf[:, t * D:(t + 1) * D],
                            mstage[:, t * D:(t + 1) * D],
                            scale[:, t:t + 1],
                        )
                    for t in range(TPP):
                        tg = piece * TPP + t
                        tp = tpsum_pool.tile([128, 128], bf16, tag="tp")
                        nc.tensor.transpose(
                            tp[:], m_bf[:, t * D:(t + 1) * D], ident[:])
                        nc.vector.tensor_copy(mT[:, tg * 128:(tg + 1) * 128], tp[:])

            # ---------------- main: sims + routed max-accumulate drains -----
            with tc.tile_pool(name="psum", bufs=4, space="PSUM") as psum_pool:
                for qt in range(NQT):
                    accs = {r: work_pool.tile([128, ACC_W], bf16,
                                              tag=f"acc{r}", name=f"acc{r}")
                            for r in ROUTE_NAMES}
                    seen = {r: False for r in ROUTE_NAMES}
                    lhsT = qT[:, qt * 128:(qt + 1) * 128]
                    for w, r, base in DRAIN_PLAN:
                        ps = psum_pool.tile([128, ACC_W], f32, tag="ps")
                        for j in range(w // 512):
                            nc.tensor.matmul(
                                ps[:, j * 512:(j + 1) * 512], lhsT,
                                mT[:, base + j * 512:base + (j + 1) * 512],
                                start=True, stop=True,
                            )
                        acc = accs[r]
                        if r == "D":
                            if not seen[r]:
                                nc.vector.tensor_copy(acc[:, :w], ps[:, :w])
                            else:
                                nc.vector.tensor_tensor(
                                    acc[:, :w], ps[:, :w], acc[:, :w], op=MAX)
                        elif not seen[r]:
                            nc.scalar.copy(acc[:, :w], ps[:, :w])
                        else:
                            tmp = work_pool.tile([128, ACC_W], bf16,
                                                 tag="tmpA", name="tmpA")
                            nc.scalar.copy(tmp[:, :w], ps[:, :w])
                            nc.vector.tensor_tensor(
                                acc[:, :w], tmp[:, :w], acc[:, :w], op=MAX)
                        seen[r] = True
                    for ri, r in enumerate(ROUTE_NAMES):
                        nc.sync.dma_start(
                            cm_out.ap()[qt * 128:(qt + 1) * 128,
                                        ri * ACC_W:(ri + 1) * ACC_W],
                            accs[r][:],
                        )

    nc.compile()
    return nc


def _identity_bf16():
    import ml_dtypes

    return np.eye(128, dtype=ml_dtypes.bfloat16)


def _in_maps(queries_np, mem_padded):
    shards = mem_padded.reshape(N_CORES, M_SHARD, D)
    ident = _identity_bf16()
    return [
        {"mem": np.ascontiguousarray(shards[c]), "queries": queries_np,
         "ident": ident}
        for c in range(N_CORES)
    ]


def _run_device(queries_np, mem_padded, trace=False):
    from concourse import bass_utils

    nc = _build_nc()
    res = bass_utils.run_bass_kernel_spmd(
        nc, _in_maps(queries_np, mem_padded),
        core_ids=list(range(N_CORES)), trace=trace,
    )
    return res


def _host_topk(queries_np, memory_np, cm_all, k):
    import ml_dtypes  # noqa: F401  (cm arrives as bfloat16)

    nq = queries_np.shape[0]
    # [NQ, N_CORES * N_ROUTES * ACC_W] routed group-max matrix
    cm = np.concatenate(
        [np.asarray(cm_all[c], dtype=np.float32) for c in range(N_CORES)], axis=1
    )
    t = min(T_GROUPS, cm.shape[1])
    top_groups = np.argpartition(-cm, t - 1, axis=1)[:, :t]  # [NQ, t]

    per_core = N_ROUTES * ACC_W
    core = top_groups // per_core
    rem = top_groups % per_core
    ri = rem // ACC_W
    u = rem % ACC_W
    members = _group_members()                       # [N_ROUTES, ACC_W, mm]
    loc = members[ri, u]                             # [NQ, t, mm]
    cand = (core[:, :, None] * M_SHARD + loc).reshape(nq, -1)
    cand = np.where(loc.reshape(nq, -1) < 0, M_PAD_TOTAL, cand)  # pad slots

    valid = cand < M_TOTAL
    cand_safe = np.where(valid, cand, 0)

    qn = queries_np / np.maximum(
        np.linalg.norm(queries_np, axis=1, keepdims=True), EPS)
    mc = memory_np[cand_safe]                             # [NQ, t*16, D]
    mc_n = np.linalg.norm(mc, axis=2, keepdims=True)
    mc = mc / np.maximum(mc_n, EPS)
    vals = np.einsum("qd,qcd->qc", qn.astype(np.float32), mc.astype(np.float32))
    vals = np.where(valid, vals, np.float32(-2.0))

    # sort candidates by index so a stable sort on -vals breaks ties by index
    ordc = np.argsort(cand_safe, axis=1)
    cand_sorted = np.take_along_axis(cand_safe, ordc, axis=1)
    vals_sorted = np.take_along_axis(vals, ordc, axis=1)
    sel = np.argsort(-vals_sorted, axis=1, kind="stable")[:, :k]

    top_vals = np.take_along_axis(vals_sorted, sel, axis=1)
    top_idx = np.take_along_axis(cand_sorted, sel, axis=1)
    distances = (np.float32(1.0) - top_vals).astype(np.float32)
    indices = top_idx.astype(np.int32)
    return distances, indices


def kernel(queries, memory, k):
    queries_np = np.ascontiguousarray(np.asarray(queries, dtype=np.float32))
    memory_np = np.ascontiguousarray(np.asarray(memory, dtype=np.float32))
    k = int(np.asarray(k))

    mem_padded = np.zeros((M_PAD_TOTAL, D), dtype=np.float32)
    mem_padded[:M_TOTAL] = memory_np

    res = _run_device(queries_np, mem_padded)
    cm_all = [res.results[c]["cm"] for c in range(N_CORES)]
    return _host_topk(queries_np, memory_np, cm_all, k)
